# revision 15
# baseline (speedup 1.0000x reference)
"""Trainium2 Bass kernel for nn_Couple_loss_62380105007762.

Loss = w0 * MSE + w1 * KLD + w2 * CE where
  sig(x)  = 2 * x[:, 0].sum(axis=F)                      (inverse SSQ-STFT, real channel only)
  MSE     = sum((sig(output_rec) - sig(target_rec))**2)
  KLD     = -0.5 * sum(1 + log_var - mean**2 - exp(log_var))
  CE      = mean cross-entropy(output_clas, target_clas)

Sharding: data-parallel over the batch dim (64 rows -> 8 cores x 8 rows).
Each core computes a weighted partial loss scalar; host sums the 8 partials
(plus the data-independent KLD constant).

Device strategy (memory-bound problem): ship the real channels as fp8 e4m3
(loss rel-err ~9e-4, gate is 2e-2), 4 MiB per core instead of 16.
  - DRAM layout is the flat-block view [128, 8, 2048]: partition p holds
    16 KB contiguous DRAM (batch row p//16, f-planes 8*(p%16)..+8), so DMA
    runs with large line-contiguous descriptors. o streams on the sync
    HWDGE queue, t on the scalar HWDGE queue, 2 x 1 MiB pieces each.
  - The host negates target_rec before fp8 conversion, so accumulating
    both tensors under the same +1 selector yields diff = sig_o - sig_t.
  - Plain fp8 matmuls, 4x column-tiled: t-chunk k -> PE column group k
    (tile_position (0, 32k)), so 4 matmuls run concurrently and PSUM
    collects diff[b, t] as [128, 512] (rows 32k + b) in a single bank.
  - ACT square + accumulate -> per-partition MSE partials -> ones-matmul
    partition reduce -> weighted dot with host-prepared w_eff -> DMA out.
  - KLD/CE computed from one packed [8, 532] f32 side tensor on DVE/ACT
    while the main stream DMAs; PE warm-up matmuls lift the HAM throttle
    before the data arrives.
"""

import numpy as np
import ml_dtypes
from contextlib import ExitStack

import concourse.bass as bass
import concourse.tile as tile
from concourse import mybir
from concourse.bass_utils import run_bass_kernel_spmd

N_CORES = 8
B, Z, F, T, C = 64, 256, 128, 2048, 5
BS = B // N_CORES   # batch rows per core
NJ = 8              # f-planes per partition line (flat-block layout)
NCHUNK = 4          # t-chunks of 512 -> 4 PE column groups
CW = T // NCHUNK    # 512 columns per chunk
NPIECE = 2          # DMA pieces per tensor (1 MiB each)
JP = NJ // NPIECE   # f-planes per piece
N_WARM = 2          # long f32 PE warm-up matmuls (HAM un-throttle)

FP32 = mybir.dt.float32
FP8 = mybir.dt.float8e4
AX = mybir.AxisListType
ALU = mybir.AluOpType
ACTF = mybir.ActivationFunctionType

# packed [8, 532] f32 side-tensor column map
SM_MEAN = slice(0, 256)
SM_LV = slice(256, 512)
SM_OC = slice(512, 517)
SM_OH = slice(517, 522)
SM_W = slice(522, 525)


def build_bass(legalize: bool = True):
    nc = bass.Bass()

    o8 = nc.declare_dram_parameter("o8", [128, NJ, T], FP8, isOutput=False)
    t8 = nc.declare_dram_parameter("t8", [128, NJ, T], FP8, isOutput=False)
    sw = nc.declare_dram_parameter("sw", [128, 32], FP8, isOutput=False)
    small = nc.declare_dram_parameter("small", [BS, 532], FP32, isOutput=False)
    out = nc.declare_dram_parameter("out", [128, 3], FP32, isOutput=True)

    with tile.TileContext(nc) as tc:
        with ExitStack() as ctx:
            sb_pool = ctx.enter_context(tc.tile_pool(name="sb", bufs=1))
            ps_pool = ctx.enter_context(tc.tile_pool(name="ps", bufs=1, space="PSUM"))
            const_pool = big_pool = small_pool = sb_pool
            psw_pool = ps_pool

            # warm-up moving source: DMA-independent (memset)
            warm_src = small_pool.tile([BS, 512], FP32, tag="wsrc")
            nc.vector.memset(warm_src[:], 0.75)

            sw_t = const_pool.tile([128, 32], FP8, tag="sw")
            small_t = small_pool.tile([BS, 532], FP32, tag="small")
            o_t = big_pool.tile([128, NJ, T], FP8, tag="o")
            t_t = big_pool.tile([128, NJ, T], FP8, tag="t")
            # queue-interleaved pieces: arrival order ~ (sw, o1, o2, t1, t2)
            # matches the matmul consumption order below.
            nc.sync.dma_start(sw_t[:], sw[:, :])
            nc.scalar.dma_start(small_t[:], small[:, :])
            nc.sync.dma_start(o_t[:, 0:JP, :], o8[:, 0:JP, :])
            nc.scalar.dma_start(o_t[:, JP:NJ, :], o8[:, JP:NJ, :])
            nc.sync.dma_start(t_t[:, 0:JP, :], t8[:, 0:JP, :])
            nc.scalar.dma_start(t_t[:, JP:NJ, :], t8[:, JP:NJ, :])

            # scratch: col0 = MSE row partials (ACT accum), col1 = KLD rows,
            # col2 = CE rows.
            scratch = small_pool.tile([128, 3], FP32, tag="scr")
            nc.vector.memset(scratch[:], 0.0)

            # ---- KLD / CE on the packed side tensor (overlaps main DMA) ----
            m_t = small_t[:, SM_MEAN]
            lv_t = small_t[:, SM_LV]
            oc_t = small_t[:, SM_OC]
            oh_t = small_t[:, SM_OH]

            msq = small_pool.tile([BS, 1], FP32, tag="msq")
            esum = small_pool.tile([BS, 1], FP32, tag="esum")
            lvsum = small_pool.tile([BS, 1], FP32, tag="lvsum")
            kl_j = small_pool.tile([BS, Z], FP32, tag="klj")
            kl_j2 = small_pool.tile([BS, Z], FP32, tag="klj2")
            kl_tmp = small_pool.tile([BS, 1], FP32, tag="kltmp")
            nc.vector.tensor_tensor(kl_j[:], m_t, m_t, ALU.mult)
            nc.vector.reduce_sum(msq[:], kl_j[:], axis=AX.X)
            nc.scalar.activation(kl_j2[:], lv_t, ACTF.Exp, accum_out=esum[:])
            nc.vector.reduce_sum(lvsum[:], lv_t, axis=AX.X)
            nc.vector.tensor_tensor(kl_tmp[:], lvsum[:], msq[:], ALU.subtract)
            nc.vector.tensor_tensor(
                scratch[0:BS, 1:2], kl_tmp[:], esum[:], ALU.subtract
            )

            rmax = small_pool.tile([BS, 1], FP32, tag="rmax")
            nmax = small_pool.tile([BS, 1], FP32, tag="nmax")
            sexp = small_pool.tile([BS, 1], FP32, tag="sexp")
            lse = small_pool.tile([BS, 1], FP32, tag="lse")
            picked = small_pool.tile([BS, 1], FP32, tag="picked")
            ce_j = small_pool.tile([BS, C], FP32, tag="cej")
            ce_j2 = small_pool.tile([BS, C], FP32, tag="cej2")
            ce_tmp = small_pool.tile([BS, 1], FP32, tag="cetmp")
            nc.vector.reduce_max(rmax[:], oc_t, axis=AX.X)
            nc.vector.tensor_scalar_mul(nmax[:], rmax[:], -1.0)
            nc.scalar.activation(
                ce_j[:], oc_t, ACTF.Exp, bias=nmax[:], accum_out=sexp[:]
            )
            nc.scalar.activation(lse[:], sexp[:], ACTF.Ln)
            nc.vector.tensor_tensor(ce_j2[:], oc_t, oh_t, ALU.mult)
            nc.vector.reduce_sum(picked[:], ce_j2[:], axis=AX.X)
            nc.vector.tensor_tensor(ce_tmp[:], rmax[:], lse[:], ALU.add)
            nc.vector.tensor_tensor(
                scratch[0:BS, 2:3], ce_tmp[:], picked[:], ALU.subtract
            )

            # ---- PE warm-up (lift HAM before the data arrives) ----
            # fp32 matmuls run 4 passes through the array (~4*N cycles of
            # sustained PE busy per instruction), so a few of them cover the
            # ~3.4us HAM window without bloating the instruction stream.
            ps_w = psw_pool.tile([1, 512], FP32, tag="psw")
            warm_ones = small_pool.tile([BS, 1], FP32, tag="wones")
            nc.vector.memset(warm_ones[:], 1.0)
            for _ in range(N_WARM):
                nc.tensor.matmul(
                    ps_w[:], warm_ones[:], warm_src[:],
                    start=True, stop=True,
                )

            # ---- main MSE stream: plain fp8, 4x column-tiled ----
            # ps[32k + b, c] accumulates diff[b, 512k + c]; column group k
            # runs concurrently with the others. The selector writes all 32
            # rows of its group (zeros beyond row 8).
            ps = ps_pool.tile([128, CW], FP32, tag="ps")
            groups = [(i, tens) for tens in ("o", "t") for i in range(NPIECE)]
            tiles = {"o": o_t, "t": t_t}
            for gi, (i, tname) in enumerate(groups):
                tens = tiles[tname]
                for j in range(JP * i, JP * (i + 1)):
                    for k in range(NCHUNK):
                        first = gi == 0 and j == JP * i
                        last = gi == len(groups) - 1 and j == JP * (i + 1) - 1
                        nc.tensor.matmul(
                            ps[32 * k:32 * k + 32, :],
                            sw_t[:],
                            tens[:, j, CW * k:CW * k + CW],
                            start=first,
                            stop=last,
                            tile_position=(0, 32 * k),
                            skip_group_check=True,
                        )

            # ---- epilogue: square-accumulate, ship the 128x3 partials ----
            # (host finishes the 384-element weighted reduce: the sharding
            # contract already sums per-shard partials host-side)
            junk = small_pool.tile([128, CW], FP32, tag="junk")
            nc.scalar.activation(
                junk[:], ps[:], ACTF.Square, accum_out=scratch[:, 0:1]
            )
            nc.scalar.dma_start(out[:, :], scratch[:])

    if legalize:
        _legalize_multi_waits(nc)
    mybir.codegen_inst_isa_subclasses(nc)
    return nc


def _legalize_multi_waits(nc):
    """walrus rejects TPB compute instructions carrying more than one sync
    wait. Hoist every wait of a multi-wait compute instruction onto
    standalone InstEventSemaphore instructions on the same engine."""
    for fn in nc.m.functions:
        for blk in fn.blocks:
            new_insts = []
            for inst in blk.instructions:
                si = inst.sync_info
                tname = type(inst).__name__
                if (
                    si is not None
                    and si.on_wait
                    and len(si.on_wait) > 1
                    and tname != "InstEventSemaphore"
                ):
                    for i, w in enumerate(si.on_wait):
                        new_insts.append(
                            mybir.InstEventSemaphore(
                                name=f"{inst.name}_hoistw{i}",
                                engine=inst.engine,
                                ins=[],
                                outs=[],
                                sync_info=mybir.SyncInfo(on_wait=[w], on_update=[]),
                            )
                        )
                    inst.sync_info = mybir.SyncInfo(
                        on_wait=[], on_update=si.on_update
                    )
                new_insts.append(inst)
            blk.instructions = new_insts


_NC_CACHE = {}


def _get_nc():
    if "nc" not in _NC_CACHE:
        _NC_CACHE["nc"] = build_bass()
    return _NC_CACHE["nc"]


def make_in_maps(inputs) -> list[dict]:
    o = np.asarray(inputs["output_rec"], dtype=np.float32)
    t = np.asarray(inputs["target_rec"], dtype=np.float32)
    mean = np.asarray(inputs["mean"], dtype=np.float32)
    log_var = np.asarray(inputs["log_var"], dtype=np.float32)
    oclas = np.asarray(inputs["output_clas"], dtype=np.float32)
    tclas = np.asarray(inputs["target_clas"]).astype(np.int64)
    w = np.asarray(inputs["weight"], dtype=np.float32).astype(np.float64)

    # Only the real channel contributes; negate target so the PE accumulates
    # sig_o - sig_t directly under one +1 selector.
    o8 = o[:, 0].astype(ml_dtypes.float8_e4m3)          # [B, F, T]
    t8 = np.negative(t[:, 0]).astype(ml_dtypes.float8_e4m3)

    onehot = np.zeros((B, C), dtype=np.float32)
    onehot[np.arange(B), tclas] = 1.0

    # selector: batch row b = p//16 -> column b (columns 8..32 zero)
    sw_np = np.zeros((128, 32), dtype=ml_dtypes.float8_e4m3)
    p = np.arange(128)
    sw_np[p, p // 16] = 1.0

    in_maps = []
    for c in range(N_CORES):
        s = slice(c * BS, (c + 1) * BS)
        small_np = np.zeros((BS, 532), dtype=np.float32)
        small_np[:, SM_MEAN] = mean[s]
        small_np[:, SM_LV] = log_var[s]
        small_np[:, SM_OC] = oclas[s]
        small_np[:, SM_OH] = onehot[s]
        in_maps.append(
            {
                # [8, 128, 2048] -> flat-block [128, 8, 2048]: partition
                # p = b*16 + f//8 holds 16 KB contiguous DRAM.
                "o8": o8[s].reshape(128, NJ, T),
                "t8": t8[s].reshape(128, NJ, T),
                "sw": sw_np,
                "small": small_np,
            }
        )
    return in_maps


def kernel(**inputs) -> np.ndarray:
    in_maps = make_in_maps(inputs)
    nc = _get_nc()
    res = run_bass_kernel_spmd(nc, in_maps, list(range(N_CORES)))
    w = np.asarray(inputs["weight"], dtype=np.float64)
    # psum of the per-shard partials: col0 = per-partition MSE row sums,
    # col1 = KLD rows, col2 = CE rows.
    parts = np.stack([np.asarray(r["out"], dtype=np.float64) for r in res.results])
    mse_s, kld_s, ce_s = parts.sum(axis=(0, 1))
    total = (
        4.0 * w[0] * mse_s                      # ISSQ scale^2 folded into w0
        + (-0.5 * w[1]) * (kld_s + B * Z)       # + data-independent KLD term
        + (w[2] / B) * ce_s
    )
    return np.float32(total)
